# revision 1
# baseline (speedup 1.0000x reference)
"""Trainium2 Bass kernel for nn_CrossAttention (single-CLS-query cross attention).

Reference computes, per batch b:
    q = x[b,0,:] @ wq.T                  (single CLS query)
    k = x[b] @ wk.T ; v = x[b] @ wv.T
    out = softmax(q k^T / sqrt(d)) v ; y = out @ wp.T + bp

Because there is a single query token, the huge K/V projections can be
eliminated algebraically:
    scores[b,h,n] = M[b,h,:] . x[b,n,:]   with  M[b,h,:] = (SCALE*q_h) @ Wk_h
    U[b,h,:]     = sum_n attn[b,h,n] x[b,n,:]
    y[b]         = concat_h(U[b,h,:] @ Wv_h.T) @ wp.T + bp
which needs only two streaming passes over x (~2.5 GMAC total) instead of
the 155 GFLOP dense projections.

Distribution: pure data parallel over batch B=32 across 8 cores (4 batches
per core), no collectives.  Each core streams its x shard twice: once in
[C, N] layout (scores, contraction over C) and once in [N, C] layout
(weighted sum, contraction over N), since the PE can only contract over the
partition dimension.  Both layouts are prepared host-side.
"""

import numpy as np

import concourse.bass as bass
import concourse.tile as tile
from concourse import bacc, mybir
from concourse.bass_utils import run_bass_kernel_spmd

# Problem constants (hardcoded per the harness contract).
B, N, C = 32, 4096, 768
H, D = 12, 64
SCALE = D ** -0.5
NCORES = 8
BSH = B // NCORES  # batches per core

F32 = mybir.dt.float32
F32R = mybir.dt.float32r

# Phase dtype knobs.  float32r runs the PE at 1 cyc/row (vs 4 for float32)
# at reduced internal precision; float32 is the conservative choice.
C_DT = F32R  # dtype for the weighted-sum (phase C) matmuls
A_DT = F32R  # dtype for the scores (phase A) matmuls
NCHUNK = C // 128  # 6
DEBUG = False


def build_kernel():
    nc = bacc.Bacc("TRN2", target_bir_lowering=False, debug=False,
                   num_devices=NCORES)

    xT = nc.dram_tensor("xT", [BSH, C, N], A_DT, kind="ExternalInput")
    x = nc.dram_tensor("x", [BSH, N, C], C_DT, kind="ExternalInput")
    x0T = nc.dram_tensor("x0T", [C, BSH], F32, kind="ExternalInput")
    wqT = nc.dram_tensor("wqT", [C, C], F32, kind="ExternalInput")
    wk = nc.dram_tensor("wk", [C, C], F32, kind="ExternalInput")
    wvT = nc.dram_tensor("wvT", [C, C], F32, kind="ExternalInput")
    wpT = nc.dram_tensor("wpT", [C, C], F32, kind="ExternalInput")
    bp = nc.dram_tensor("bp", [1, C], F32, kind="ExternalInput")
    i12 = nc.dram_tensor("i12", [H, H], F32, kind="ExternalInput")
    y = nc.dram_tensor("y", [BSH, C], F32, kind="ExternalOutput")
    dbg = {}
    if DEBUG:
        dbg["qT"] = nc.dram_tensor("dbg_qT", [128, NCHUNK, BSH], F32,
                                   kind="ExternalOutput").ap()
        dbg["mT"] = nc.dram_tensor("dbg_mT", [128, NCHUNK, BSH, H], F32,
                                   kind="ExternalOutput").ap()
        dbg["attnT"] = nc.dram_tensor("dbg_attnT", [128, N // 128, H], F32,
                                      kind="ExternalOutput").ap()
        dbg["U"] = nc.dram_tensor("dbg_U", [H, C], F32,
                                  kind="ExternalOutput").ap()

    with tile.TileContext(nc) as tc:
        cross_attn_kernel(tc, y.ap(), xT.ap(), x.ap(), x0T.ap(), wqT.ap(),
                          wk.ap(), wvT.ap(), wpT.ap(), bp.ap(), i12.ap(), dbg)
    nc.compile()
    return nc


def cross_attn_kernel(tc, y, xT, x, x0T, wqT, wk, wvT, wpT, bp, i12, dbg={}):
    from contextlib import ExitStack
    ctx = ExitStack()
    nc = tc.nc
    with ctx:
        consts = ctx.enter_context(tc.tile_pool(name="consts", bufs=1))
        xa_pool = ctx.enter_context(tc.tile_pool(name="xa", bufs=20))
        xc_pool = ctx.enter_context(tc.tile_pool(name="xc", bufs=12))
        attn_pool = ctx.enter_context(tc.tile_pool(name="attn", bufs=2))
        small = ctx.enter_context(tc.tile_pool(name="small", bufs=2))
        ps_a = ctx.enter_context(tc.tile_pool(name="ps_a", bufs=2, space="PSUM"))
        ps_c = ctx.enter_context(tc.tile_pool(name="ps_c", bufs=1, space="PSUM"))
        ps_misc = ctx.enter_context(tc.tile_pool(name="ps_misc", bufs=2, space="PSUM"))

        # ---- constant loads ----
        # All on the scalar HWDGE queue so the sync queue starts streaming
        # x tiles immediately; wvT/wpT are deferred until P4 needs them.
        def load_w(ap_dram, name):
            t = consts.tile([128, NCHUNK, C], F32, tag=name)
            nc.scalar.dma_start(out=t, in_=ap_dram.rearrange("(a p) o -> p a o", p=128))
            return t

        wqT_sb = load_w(wqT, "wqT_sb")
        wk_sb = load_w(wk, "wk_sb")
        x0T_sb = consts.tile([128, NCHUNK, BSH], F32)
        nc.scalar.dma_start(out=x0T_sb, in_=x0T.rearrange("(a p) b -> p a b", p=128))
        i12_sb = consts.tile([H, H], F32)
        nc.scalar.dma_start(out=i12_sb, in_=i12)
        bp_sb = consts.tile([BSH, C], F32)
        nc.scalar.dma_start(
            out=bp_sb,
            in_=bass.AP(tensor=bp.tensor, offset=0, ap=[[0, BSH], [1, C]]),
        )
        qT_sb = consts.tile([128, NCHUNK, BSH], F32)
        # written by a casting tensor_copy from f32 PSUM, read by phase-A matmul
        mT_sb = consts.tile([128, NCHUNK, BSH, H], A_DT)

        # ---- P0a: qT[c_out, b] = wq @ (SCALE * x0^T), contraction over c_in ----
        for co in range(NCHUNK):
            ps_q = ps_misc.tile([128, BSH], F32, tag="misc")
            for ci in range(NCHUNK):
                nc.tensor.matmul(
                    ps_q,
                    lhsT=wqT_sb[:, ci, co * 128:(co + 1) * 128],
                    rhs=x0T_sb[:, ci, :],
                    start=(ci == 0), stop=(ci == NCHUNK - 1),
                )
            nc.vector.tensor_copy(qT_sb[:, co, :], ps_q)

        # ---- P0b: mT[c, b, h] = Wk_h^T @ qT_h  (contraction over d=64) ----
        for ci in range(NCHUNK):
            for h in range(H):
                po = (h % 2) * 64
                ch = h // 2
                ps_m = ps_misc.tile([128, BSH], F32, tag="misc")
                nc.tensor.matmul(
                    ps_m,
                    lhsT=wk_sb[po:po + 64, ch, ci * 128:(ci + 1) * 128],
                    rhs=qT_sb[po:po + 64, ch, :],
                    start=True, stop=True,
                )
                nc.vector.tensor_copy(mT_sb[:, ci, :, h], ps_m)

        ut_all = consts.tile([128, NCHUNK, BSH, H], F32)  # U^T[c, b, h]
        if dbg:
            nc.sync.dma_start(out=dbg["qT"], in_=qT_sb)
            nc.sync.dma_start(out=dbg["mT"], in_=mT_sb)

        # ---- per-batch main loop ----
        for b in range(BSH):
            # phase A: scores[h, n] = sum_c mT[c, h] * xT[c, n]; exp is fused
            # into the PSUM->SBUF move (no max subtraction needed: |scores|<8)
            attn = attn_pool.tile([H, N], F32, tag="attn")
            partials = small.tile([H, N // 512], F32, tag="partials")
            for nt in range(N // 512):
                xa = []
                for ci in range(NCHUNK):
                    t = xa_pool.tile([128, 512], A_DT, tag="xa")
                    nc.sync.dma_start(
                        out=t,
                        in_=xT[b, ci * 128:(ci + 1) * 128, nt * 512:(nt + 1) * 512],
                    )
                    xa.append(t)
                ps = ps_a.tile([H, 512], F32, tag="psA")
                for ci in range(NCHUNK):
                    nc.tensor.matmul(
                        ps,
                        lhsT=mT_sb[:, ci, b, :],
                        rhs=xa[ci],
                        start=(ci == 0), stop=(ci == NCHUNK - 1),
                    )
                nc.scalar.activation(
                    out=attn[:, nt * 512:(nt + 1) * 512], in_=ps,
                    func=mybir.ActivationFunctionType.Exp,
                    accum_out=partials[:, nt:nt + 1],
                )

            sums = small.tile([H, 1], F32, tag="sums")
            nc.vector.reduce_sum(sums, partials, axis=mybir.AxisListType.X)
            rsum = small.tile([H, 1], F32, tag="rsum")
            nc.vector.reciprocal(rsum, sums)

            # transpose attn -> attnT[n, h] chunks (PE transpose via identity);
            # the PSUM->SBUF copy also casts to the phase-C matmul dtype
            attnT = attn_pool.tile([128, N // 128, H], C_DT, tag="attnT")
            for nn in range(N // 128):
                ps_t = ps_a.tile([128, H], F32, tag="psAT")
                nc.tensor.transpose(
                    ps_t, in_=attn[:, nn * 128:(nn + 1) * 128], identity=i12_sb)
                nc.vector.tensor_copy(attnT[:, nn, :], ps_t)
            if dbg and b == 0:
                nc.sync.dma_start(out=dbg["attnT"], in_=attnT)

            # phase C: U[h, c] = sum_n attnT[n, h] * x[n, c]
            psU0 = ps_c.tile([H, 384], F32, tag="psC0")
            psU1 = ps_c.tile([H, 384], F32, tag="psC1")
            psU = [psU0, psU1]
            for nn in range(N // 128):
                xc = xc_pool.tile([128, C], C_DT, tag="xc")
                # issue phase-C loads on the other HWDGE engine so the two
                # x streams ride independent DMA queues
                nc.scalar.dma_start(out=xc, in_=x[b, nn * 128:(nn + 1) * 128, :])
                for j in range(2):
                    nc.tensor.matmul(
                        psU[j],
                        lhsT=attnT[:, nn, :],
                        rhs=xc[:, j * 384:(j + 1) * 384],
                        start=(nn == 0), stop=(nn == N // 128 - 1),
                    )
            # normalize by softmax sum while moving PSUM -> SBUF
            U_sb = small.tile([H, C], F32, tag="U")
            for j in range(2):
                nc.vector.tensor_scalar_mul(
                    out=U_sb[:, j * 384:(j + 1) * 384], in0=psU[j], scalar1=rsum,
                )

            if dbg and b == 0:
                nc.sync.dma_start(out=dbg["U"], in_=U_sb)
            # transpose U -> UT[c, h] chunks for the output projections
            for k in range(NCHUNK):
                ps_t = ps_misc.tile([128, H], F32, tag="misc")
                nc.tensor.transpose(ps_t, in_=U_sb[:, k * 128:(k + 1) * 128],
                                    identity=i12_sb)
                nc.vector.tensor_copy(ut_all[:, k, b, :], ps_t)

        # ---- P4a: ypre[h*64+d, b] = sum_c wvT[c, h*64+d] * UT[c, b, h] ----
        # these ride the sync queue, which is idle after the last xa tile
        wvT_sb = consts.tile([128, NCHUNK, C], F32, tag="wvT_sb")
        nc.sync.dma_start(out=wvT_sb, in_=wvT.rearrange("(a p) o -> p a o", p=128))
        wpT_sb = consts.tile([128, NCHUNK, C], F32, tag="wpT_sb")
        nc.sync.dma_start(out=wpT_sb, in_=wpT.rearrange("(a p) o -> p a o", p=128))
        ypT_sb = consts.tile([128, NCHUNK, BSH], F32)
        for h in range(H):
            ps_yp = ps_misc.tile([64, BSH], F32, tag="misc")
            for k in range(NCHUNK):
                nc.tensor.matmul(
                    ps_yp,
                    lhsT=wvT_sb[:, k, h * 64:(h + 1) * 64],
                    rhs=ut_all[:, k, :, h],
                    start=(k == 0), stop=(k == NCHUNK - 1),
                )
            po = (h % 2) * 64
            nc.vector.tensor_copy(ypT_sb[po:po + 64, h // 2, :], ps_yp)

        # ---- P4b: y[b, c_out] = sum_c ypT[c, b] * wpT[c, c_out] + bp ----
        y_sb = small.tile([BSH, C], F32, tag="y")
        for j in range(2):
            ps_y = ps_misc.tile([BSH, 384], F32, tag="misc")
            for k in range(NCHUNK):
                nc.tensor.matmul(
                    ps_y,
                    lhsT=ypT_sb[:, k, :],
                    rhs=wpT_sb[:, k, j * 384:(j + 1) * 384],
                    start=(k == 0), stop=(k == NCHUNK - 1),
                )
            nc.vector.tensor_add(
                out=y_sb[:, j * 384:(j + 1) * 384],
                in0=ps_y,
                in1=bp_sb[:, j * 384:(j + 1) * 384],
            )
        nc.sync.dma_start(out=y, in_=y_sb)


_CACHE = {}


def kernel(x, wq, wk, wv, wp, bp, trace=False):
    x = np.ascontiguousarray(x, dtype=np.float32)
    wq = np.asarray(wq, dtype=np.float32)
    wk = np.asarray(wk, dtype=np.float32)
    wv = np.asarray(wv, dtype=np.float32)
    wp = np.asarray(wp, dtype=np.float32)
    bp = np.asarray(bp, dtype=np.float32)

    if "nc" not in _CACHE:
        _CACHE["nc"] = build_kernel()
    nc = _CACHE["nc"]

    x_sh = x.reshape(NCORES, BSH, N, C)
    wqT = np.ascontiguousarray(wq.T)
    wkn = np.ascontiguousarray(wk)
    wvT = np.ascontiguousarray(wv.T)
    wpT = np.ascontiguousarray(wp.T)
    bp2 = np.ascontiguousarray(bp.reshape(1, C))
    i12 = np.eye(H, dtype=np.float32)

    in_maps = []
    for k in range(NCORES):
        xs = x_sh[k]
        in_maps.append({
            "xT": np.ascontiguousarray(xs.transpose(0, 2, 1)),
            "x": np.ascontiguousarray(xs),
            "x0T": np.ascontiguousarray((xs[:, 0, :] * SCALE).T),
            "wqT": wqT,
            "wk": wkn,
            "wvT": wvT,
            "wpT": wpT,
            "bp": bp2,
            "i12": i12,
        })

    res = run_bass_kernel_spmd(nc, in_maps, core_ids=list(range(NCORES)),
                               trace=trace)
    out = np.concatenate([res.results[k]["y"] for k in range(NCORES)], axis=0)
    out = out.reshape(B, 1, C)
    if trace:
        _CACHE["last_exec_time_ns"] = res.exec_time_ns
        _CACHE["last_results"] = res
    return out



# revision 3
# speedup vs baseline: 1.8961x; 1.8961x over previous
"""Trainium2 Bass kernel for nn_CrossAttention (single-CLS-query cross attention).

Reference computes, per batch b:
    q = x[b,0,:] @ wq.T                  (single CLS query)
    k = x[b] @ wk.T ; v = x[b] @ wv.T
    out = softmax(q k^T / sqrt(d)) v ; y = out @ wp.T + bp

Because there is a single query token, the huge K/V projections can be
eliminated algebraically:
    scores[b,h,n] = M[b,h,:] . x[b,n,:]   with  M[b,h,:] = (SCALE*q_h) @ Wk_h
    U[b,h,:]     = sum_n attn[b,h,n] x[b,n,:]
    y[b]         = concat_h(U[b,h,:] @ Wv_h.T) @ wp.T + bp
which needs only two streaming passes over x (~2.5 GMAC total) instead of
the 155 GFLOP dense projections.

Distribution: pure data parallel over batch B=32 across 8 cores (4 batches
per core), no collectives.  Each core streams its x shard twice: once in
[C, N] layout (scores, contraction over C) and once in [N, C] layout
(weighted sum, contraction over N), since the PE can only contract over the
partition dimension.  Both layouts are prepared host-side in bfloat16, so
the two passes together cost the same HBM traffic as a single fp32 pass.
"""

import numpy as np

import concourse.bass as bass
import concourse.tile as tile
from concourse import bacc, mybir
from concourse.bass_utils import run_bass_kernel_spmd

# Problem constants (hardcoded per the harness contract).
B, N, C = 32, 4096, 768
H, D = 12, 64
SCALE = D ** -0.5
NCORES = 8
BSH = B // NCORES  # batches per core

F32 = mybir.dt.float32
BF16 = mybir.dt.bfloat16

NCHUNK = C // 128  # 6
NTW = 1024         # phase-A n-window per DMA
NCW = 4            # phase-C 128-row n-chunks per DMA


def build_kernel():
    nc = bacc.Bacc("TRN2", target_bir_lowering=False, debug=False,
                   num_devices=NCORES)

    xT = nc.dram_tensor("xT", [BSH, C, N], BF16, kind="ExternalInput")
    x = nc.dram_tensor("x", [BSH, N, C], BF16, kind="ExternalInput")
    x0T = nc.dram_tensor("x0T", [C, BSH], BF16, kind="ExternalInput")
    wqT = nc.dram_tensor("wqT", [C, C], BF16, kind="ExternalInput")
    wk = nc.dram_tensor("wk", [C, C], BF16, kind="ExternalInput")
    wvT = nc.dram_tensor("wvT", [C, C], BF16, kind="ExternalInput")
    wpT = nc.dram_tensor("wpT", [C, C], BF16, kind="ExternalInput")
    bp = nc.dram_tensor("bp", [1, C], F32, kind="ExternalInput")
    i12 = nc.dram_tensor("i12", [H, H], F32, kind="ExternalInput")
    y = nc.dram_tensor("y", [BSH, C], F32, kind="ExternalOutput")

    with tile.TileContext(nc) as tc:
        cross_attn_kernel(tc, y.ap(), xT.ap(), x.ap(), x0T.ap(), wqT.ap(),
                          wk.ap(), wvT.ap(), wpT.ap(), bp.ap(), i12.ap())
    nc.compile()
    return nc


def cross_attn_kernel(tc, y, xT, x, x0T, wqT, wk, wvT, wpT, bp, i12):
    from contextlib import ExitStack
    ctx = ExitStack()
    nc = tc.nc
    with ctx:
        consts = ctx.enter_context(tc.tile_pool(name="consts", bufs=1))
        xa_pool = ctx.enter_context(tc.tile_pool(name="xa", bufs=5))
        xc_pool = ctx.enter_context(tc.tile_pool(name="xc", bufs=5))
        attn_pool = ctx.enter_context(tc.tile_pool(name="attn", bufs=2))
        small = ctx.enter_context(tc.tile_pool(name="small", bufs=2))
        ps_a = ctx.enter_context(tc.tile_pool(name="ps_a", bufs=2, space="PSUM"))
        ps_c = ctx.enter_context(tc.tile_pool(name="ps_c", bufs=1, space="PSUM"))
        ps_misc = ctx.enter_context(tc.tile_pool(name="ps_misc", bufs=2, space="PSUM"))

        # ---- constant loads ----
        # All on the scalar HWDGE queue so the sync queue starts streaming
        # x tiles immediately; wvT/wpT are deferred until P4 needs them.
        def load_w(ap_dram, name):
            t = consts.tile([128, NCHUNK, C], BF16, tag=name)
            nc.scalar.dma_start(out=t, in_=ap_dram.rearrange("(a p) o -> p a o", p=128))
            return t

        wqT_sb = load_w(wqT, "wqT_sb")
        wk_sb = load_w(wk, "wk_sb")
        x0T_sb = consts.tile([128, NCHUNK, BSH], BF16)
        nc.scalar.dma_start(out=x0T_sb, in_=x0T.rearrange("(a p) b -> p a b", p=128))
        i12_sb = consts.tile([H, H], F32)
        nc.scalar.dma_start(out=i12_sb, in_=i12)
        bp_sb = consts.tile([BSH, C], F32)
        nc.scalar.dma_start(
            out=bp_sb,
            in_=bass.AP(tensor=bp.tensor, offset=0, ap=[[0, BSH], [1, C]]),
        )
        qT_sb = consts.tile([128, NCHUNK, BSH], BF16)
        # written by a casting tensor_copy from f32 PSUM, read by phase-A matmul
        mT_sb = consts.tile([128, NCHUNK, BSH, H], BF16)

        # ---- P0a: qT[c_out, b] = wq @ (SCALE * x0^T), contraction over c_in ----
        for co in range(NCHUNK):
            ps_q = ps_misc.tile([128, BSH], F32, tag="misc")
            for ci in range(NCHUNK):
                nc.tensor.matmul(
                    ps_q,
                    lhsT=wqT_sb[:, ci, co * 128:(co + 1) * 128],
                    rhs=x0T_sb[:, ci, :],
                    start=(ci == 0), stop=(ci == NCHUNK - 1),
                )
            nc.vector.tensor_copy(qT_sb[:, co, :], ps_q)

        # ---- P0b: mT[c, b, h] = Wk_h^T @ qT_h  (contraction over d=64) ----
        for ci in range(NCHUNK):
            for h in range(H):
                po = (h % 2) * 64
                ch = h // 2
                ps_m = ps_misc.tile([128, BSH], F32, tag="misc")
                nc.tensor.matmul(
                    ps_m,
                    lhsT=wk_sb[po:po + 64, ch, ci * 128:(ci + 1) * 128],
                    rhs=qT_sb[po:po + 64, ch, :],
                    start=True, stop=True,
                )
                nc.vector.tensor_copy(mT_sb[:, ci, :, h], ps_m)

        ut_all = consts.tile([128, NCHUNK, BSH, H], BF16)  # U^T[c, b, h]

        # ---- per-batch main loop ----
        for b in range(BSH):
            # phase A: scores[h, n] = sum_c mT[c, h] * xT[c, n]; exp is fused
            # into the PSUM->SBUF move (no max subtraction needed: |scores|<8)
            attn = attn_pool.tile([H, N], F32, tag="attn")
            partials = small.tile([H, N // 512], F32, tag="partials")
            for nt in range(N // NTW):
                xa = xa_pool.tile([128, NCHUNK, NTW], BF16, tag="xa")
                nc.sync.dma_start(
                    out=xa,
                    in_=xT[b].rearrange("(a p) n -> p a n", p=128)
                         [:, :, nt * NTW:(nt + 1) * NTW],
                )
                for s in range(NTW // 512):
                    n0 = nt * NTW + s * 512
                    ps = ps_a.tile([H, 512], F32, tag="psA")
                    for ci in range(NCHUNK):
                        nc.tensor.matmul(
                            ps,
                            lhsT=mT_sb[:, ci, b, :],
                            rhs=xa[:, ci, s * 512:(s + 1) * 512],
                            start=(ci == 0), stop=(ci == NCHUNK - 1),
                        )
                    nc.scalar.activation(
                        out=attn[:, n0:n0 + 512], in_=ps,
                        func=mybir.ActivationFunctionType.Exp,
                        accum_out=partials[:, n0 // 512:n0 // 512 + 1],
                    )

            sums = small.tile([H, 1], F32, tag="sums")
            nc.vector.reduce_sum(sums, partials, axis=mybir.AxisListType.X)
            rsum = small.tile([H, 1], F32, tag="rsum")
            nc.vector.reciprocal(rsum, sums)

            # transpose attn -> attnT[n, h] chunks (PE transpose via identity);
            # the PSUM->SBUF copy also casts to bf16 for the phase-C matmul
            attnT = attn_pool.tile([128, N // 128, H], BF16, tag="attnT")
            for nn in range(N // 128):
                ps_t = ps_a.tile([128, H], F32, tag="psAT")
                nc.tensor.transpose(
                    ps_t, in_=attn[:, nn * 128:(nn + 1) * 128], identity=i12_sb)
                nc.vector.tensor_copy(attnT[:, nn, :], ps_t)

            # phase C: U[h, c] = sum_n attnT[n, h] * x[n, c]
            psU0 = ps_c.tile([H, 384], F32, tag="psC0")
            psU1 = ps_c.tile([H, 384], F32, tag="psC1")
            psU = [psU0, psU1]
            for nw in range(N // (128 * NCW)):
                xc = xc_pool.tile([128, NCW, C], BF16, tag="xc")
                # issue phase-C loads on the other HWDGE engine so the two
                # x streams ride independent DMA queues
                nc.scalar.dma_start(
                    out=xc,
                    in_=x[b].rearrange("(t p) c -> p t c", p=128)
                         [:, nw * NCW:(nw + 1) * NCW, :],
                )
                for t in range(NCW):
                    nn = nw * NCW + t
                    for j in range(2):
                        nc.tensor.matmul(
                            psU[j],
                            lhsT=attnT[:, nn, :],
                            rhs=xc[:, t, j * 384:(j + 1) * 384],
                            start=(nn == 0), stop=(nn == N // 128 - 1),
                        )
            # normalize by softmax sum while moving PSUM -> SBUF
            U_sb = small.tile([H, C], F32, tag="U")
            for j in range(2):
                nc.vector.tensor_scalar_mul(
                    out=U_sb[:, j * 384:(j + 1) * 384], in0=psU[j], scalar1=rsum,
                )

            # transpose U -> UT[c, h] chunks for the output projections
            for k in range(NCHUNK):
                ps_t = ps_misc.tile([128, H], F32, tag="misc")
                nc.tensor.transpose(ps_t, in_=U_sb[:, k * 128:(k + 1) * 128],
                                    identity=i12_sb)
                nc.vector.tensor_copy(ut_all[:, k, b, :], ps_t)

        # ---- P4a: ypre[h*64+d, b] = sum_c wvT[c, h*64+d] * UT[c, b, h] ----
        # these ride the sync queue, which is idle after the last xa tile
        wvT_sb = consts.tile([128, NCHUNK, C], BF16, tag="wvT_sb")
        nc.sync.dma_start(out=wvT_sb, in_=wvT.rearrange("(a p) o -> p a o", p=128))
        wpT_sb = consts.tile([128, NCHUNK, C], BF16, tag="wpT_sb")
        nc.sync.dma_start(out=wpT_sb, in_=wpT.rearrange("(a p) o -> p a o", p=128))
        ypT_sb = consts.tile([128, NCHUNK, BSH], BF16)
        for h in range(H):
            ps_yp = ps_misc.tile([64, BSH], F32, tag="misc")
            for k in range(NCHUNK):
                nc.tensor.matmul(
                    ps_yp,
                    lhsT=wvT_sb[:, k, h * 64:(h + 1) * 64],
                    rhs=ut_all[:, k, :, h],
                    start=(k == 0), stop=(k == NCHUNK - 1),
                )
            po = (h % 2) * 64
            nc.vector.tensor_copy(ypT_sb[po:po + 64, h // 2, :], ps_yp)

        # ---- P4b: y[b, c_out] = sum_c ypT[c, b] * wpT[c, c_out] + bp ----
        y_sb = small.tile([BSH, C], F32, tag="y")
        for j in range(2):
            ps_y = ps_misc.tile([BSH, 384], F32, tag="misc")
            for k in range(NCHUNK):
                nc.tensor.matmul(
                    ps_y,
                    lhsT=ypT_sb[:, k, :],
                    rhs=wpT_sb[:, k, j * 384:(j + 1) * 384],
                    start=(k == 0), stop=(k == NCHUNK - 1),
                )
            nc.vector.tensor_add(
                out=y_sb[:, j * 384:(j + 1) * 384],
                in0=ps_y,
                in1=bp_sb[:, j * 384:(j + 1) * 384],
            )
        nc.sync.dma_start(out=y, in_=y_sb)


_CACHE = {}
_BF16 = mybir.dt.np(mybir.dt.bfloat16)


def kernel(x, wq, wk, wv, wp, bp, trace=False):
    x = np.ascontiguousarray(x, dtype=np.float32)
    wq = np.asarray(wq, dtype=np.float32)
    wk = np.asarray(wk, dtype=np.float32)
    wv = np.asarray(wv, dtype=np.float32)
    wp = np.asarray(wp, dtype=np.float32)
    bp = np.asarray(bp, dtype=np.float32)

    if "nc" not in _CACHE:
        _CACHE["nc"] = build_kernel()
    nc = _CACHE["nc"]

    x_sh = x.reshape(NCORES, BSH, N, C)
    wqT = np.ascontiguousarray(wq.T.astype(_BF16))
    wkn = np.ascontiguousarray(wk.astype(_BF16))
    wvT = np.ascontiguousarray(wv.T.astype(_BF16))
    wpT = np.ascontiguousarray(wp.T.astype(_BF16))
    bp2 = np.ascontiguousarray(bp.reshape(1, C))
    i12 = np.eye(H, dtype=np.float32)

    in_maps = []
    for k in range(NCORES):
        xs = x_sh[k]
        in_maps.append({
            "xT": np.ascontiguousarray(xs.transpose(0, 2, 1).astype(_BF16)),
            "x": np.ascontiguousarray(xs.astype(_BF16)),
            "x0T": np.ascontiguousarray((xs[:, 0, :] * SCALE).T.astype(_BF16)),
            "wqT": wqT,
            "wk": wkn,
            "wvT": wvT,
            "wpT": wpT,
            "bp": bp2,
            "i12": i12,
        })

    res = run_bass_kernel_spmd(nc, in_maps, core_ids=list(range(NCORES)),
                               trace=trace)
    out = np.concatenate([res.results[k]["y"] for k in range(NCORES)], axis=0)
    out = out.reshape(B, 1, C).astype(np.float32)
    if trace:
        _CACHE["last_exec_time_ns"] = res.exec_time_ns
        _CACHE["last_results"] = res
    return out


# revision 5
# speedup vs baseline: 2.0529x; 1.0827x over previous
"""Trainium2 Bass kernel for nn_CrossAttention (single-CLS-query cross attention).

Reference computes, per batch b:
    q = x[b,0,:] @ wq.T                  (single CLS query)
    k = x[b] @ wk.T ; v = x[b] @ wv.T
    out = softmax(q k^T / sqrt(d)) v ; y = out @ wp.T + bp

Because there is a single query token, the huge K/V projections can be
eliminated algebraically:
    scores[b,h,n] = M[b,h,:] . x[b,n,:]   with  M[b,h,:] = (SCALE*q_h) @ Wk_h
    U[b,h,:]     = sum_n attn[b,h,n] x[b,n,:]
    y[b]         = concat_h(U[b,h,:] @ Wv_h.T) @ wp.T + bp
which needs only two streaming passes over x (~2.5 GMAC total) instead of
the 155 GFLOP dense projections.

Distribution: pure data parallel over batch B=32 across 8 cores (4 batches
per core), no collectives.  The scores need x in [C, N] layout (contraction
over C on the PE partition axis) and the weighted sum needs [N, C]; both
operands stream in bfloat16.  Half of each batch's [N, C] data is produced
on-chip by PE-transposing the already-resident [C, N] tiles (bf16 PSUM out,
one copy per 128-row chunk, alternating DVE/ACT), so HBM only carries
1.5 passes over x instead of 2.
"""

import numpy as np

import concourse.bass as bass
import concourse.tile as tile
from concourse import bacc, mybir
from concourse.bass_utils import run_bass_kernel_spmd

# Problem constants (hardcoded per the harness contract).
B, N, C = 32, 4096, 768
H, D = 12, 64
SCALE = D ** -0.5
NCORES = 8
BSH = B // NCORES  # batches per core

F32 = mybir.dt.float32
BF16 = mybir.dt.bfloat16

NCHUNK = C // 128   # 6
NTW = 1024          # phase-A n-window per DMA
NWIN = N // NTW     # 4 windows per batch
CPW = NTW // 128    # 8 n-chunks per window
TPW = 4             # chunks per window transposed on-chip (rest DMA'd)
TCH = NWIN * TPW    # transposed chunks per batch
DPW = CPW - TPW     # chunks per window DMA'd


def build_kernel():
    nc = bacc.Bacc("TRN2", target_bir_lowering=False, debug=False,
                   num_devices=NCORES)

    xT = nc.dram_tensor("xT", [BSH, C, N], BF16, kind="ExternalInput")
    x = nc.dram_tensor("x", [BSH, N, C], BF16, kind="ExternalInput")
    x0T = nc.dram_tensor("x0T", [C, BSH], BF16, kind="ExternalInput")
    wqT = nc.dram_tensor("wqT", [C, C], BF16, kind="ExternalInput")
    wk = nc.dram_tensor("wk", [C, C], BF16, kind="ExternalInput")
    wvT = nc.dram_tensor("wvT", [C, C], BF16, kind="ExternalInput")
    wpT = nc.dram_tensor("wpT", [C, C], BF16, kind="ExternalInput")
    bp = nc.dram_tensor("bp", [1, C], F32, kind="ExternalInput")
    i12 = nc.dram_tensor("i12", [H, H], F32, kind="ExternalInput")
    i128 = nc.dram_tensor("i128", [128, 128], BF16, kind="ExternalInput")
    y = nc.dram_tensor("y", [BSH, C], F32, kind="ExternalOutput")

    with tile.TileContext(nc) as tc:
        cross_attn_kernel(tc, y.ap(), xT.ap(), x.ap(), x0T.ap(), wqT.ap(),
                          wk.ap(), wvT.ap(), wpT.ap(), bp.ap(), i12.ap(),
                          i128.ap())
    nc.compile()
    return nc


def cross_attn_kernel(tc, y, xT, x, x0T, wqT, wk, wvT, wpT, bp, i12, i128):
    from contextlib import ExitStack
    ctx = ExitStack()
    nc = tc.nc
    with ctx:
        consts = ctx.enter_context(tc.tile_pool(name="consts", bufs=1))
        xa_pool = ctx.enter_context(tc.tile_pool(name="xa", bufs=4))
        xct_pool = ctx.enter_context(tc.tile_pool(name="xct", bufs=2))
        xc_pool = ctx.enter_context(tc.tile_pool(name="xc", bufs=4))
        attn_pool = ctx.enter_context(tc.tile_pool(name="attn", bufs=2))
        small = ctx.enter_context(tc.tile_pool(name="small", bufs=2))
        ps_a = ctx.enter_context(tc.tile_pool(name="ps_a", bufs=2, space="PSUM"))
        ps_x = ctx.enter_context(tc.tile_pool(name="ps_x", bufs=2, space="PSUM"))
        ps_c = ctx.enter_context(tc.tile_pool(name="ps_c", bufs=1, space="PSUM"))
        ps_misc = ctx.enter_context(tc.tile_pool(name="ps_misc", bufs=2, space="PSUM"))

        # ---- constant loads ----
        # All on the scalar HWDGE queue so the sync queue starts streaming
        # x tiles immediately; wvT/wpT are deferred until P4 needs them.
        def load_w(ap_dram, name):
            t = consts.tile([128, NCHUNK, C], BF16, tag=name)
            nc.scalar.dma_start(out=t, in_=ap_dram.rearrange("(a p) o -> p a o", p=128))
            return t

        wqT_sb = load_w(wqT, "wqT_sb")
        wk_sb = load_w(wk, "wk_sb")
        x0T_sb = consts.tile([128, NCHUNK, BSH], BF16)
        nc.scalar.dma_start(out=x0T_sb, in_=x0T.rearrange("(a p) b -> p a b", p=128))
        i12_sb = consts.tile([H, H], F32)
        nc.scalar.dma_start(out=i12_sb, in_=i12)
        i128_sb = consts.tile([128, 128], BF16)
        nc.scalar.dma_start(out=i128_sb, in_=i128)
        bp_sb = consts.tile([BSH, C], F32)
        nc.scalar.dma_start(
            out=bp_sb,
            in_=bass.AP(tensor=bp.tensor, offset=0, ap=[[0, BSH], [1, C]]),
        )
        qT_sb = consts.tile([128, NCHUNK, BSH], BF16)
        # written by a casting tensor_copy from f32 PSUM, read by phase-A matmul
        mT_sb = consts.tile([128, NCHUNK, BSH, H], BF16)

        # ---- P0a: qT[c_out, b] = wq @ (SCALE * x0^T), contraction over c_in ----
        for co in range(NCHUNK):
            ps_q = ps_misc.tile([128, BSH], F32, tag="misc")
            for ci in range(NCHUNK):
                nc.tensor.matmul(
                    ps_q,
                    lhsT=wqT_sb[:, ci, co * 128:(co + 1) * 128],
                    rhs=x0T_sb[:, ci, :],
                    start=(ci == 0), stop=(ci == NCHUNK - 1),
                )
            nc.vector.tensor_copy(qT_sb[:, co, :], ps_q)

        # ---- P0b: mT[c, b, h] = Wk_h^T @ qT_h  (contraction over d=64) ----
        for ci in range(NCHUNK):
            for h in range(H):
                po = (h % 2) * 64
                ch = h // 2
                ps_m = ps_misc.tile([128, BSH], F32, tag="misc")
                nc.tensor.matmul(
                    ps_m,
                    lhsT=wk_sb[po:po + 64, ch, ci * 128:(ci + 1) * 128],
                    rhs=qT_sb[po:po + 64, ch, :],
                    start=True, stop=True,
                )
                nc.vector.tensor_copy(mT_sb[:, ci, :, h], ps_m)

        ut_all = consts.tile([128, NCHUNK, BSH, H], BF16)  # U^T[c, b, h]

        # ---- per-batch main loop ----
        for b in range(BSH):
            # phase A: scores[h, n] = sum_c mT[c, h] * xT[c, n]; exp is fused
            # into the PSUM->SBUF move (no max subtraction needed: |scores|<8).
            # The first TPW 128-row chunks of each window are also PE-transposed
            # into xcT (the [N, C] layout) so phase C doesn't re-stream them.
            attn = attn_pool.tile([H, N], F32, tag="attn")
            xcT = xct_pool.tile([128, TCH, C], BF16, tag="xcT")
            partials = small.tile([H, N // 512], F32, tag="partials")
            for w in range(NWIN):
                xa = xa_pool.tile([128, NCHUNK, NTW], BF16, tag="xa")
                nc.sync.dma_start(
                    out=xa,
                    in_=xT[b].rearrange("(a p) n -> p a n", p=128)
                         [:, :, w * NTW:(w + 1) * NTW],
                )
                for s in range(NTW // 512):
                    n0 = w * NTW + s * 512
                    ps = ps_a.tile([H, 512], F32, tag="psA")
                    for ci in range(NCHUNK):
                        nc.tensor.matmul(
                            ps,
                            lhsT=mT_sb[:, ci, b, :],
                            rhs=xa[:, ci, s * 512:(s + 1) * 512],
                            start=(ci == 0), stop=(ci == NCHUNK - 1),
                        )
                    nc.scalar.activation(
                        out=attn[:, n0:n0 + 512], in_=ps,
                        func=mybir.ActivationFunctionType.Exp,
                        accum_out=partials[:, n0 // 512:n0 // 512 + 1],
                    )
                # on-chip transpose of chunks [0, TPW) of this window into the
                # [N, C] layout (bf16 PSUM out, one copy per chunk)
                for l in range(TPW):
                    t = w * TPW + l
                    ps_xt = ps_x.tile([128, C], BF16, tag="psx")
                    for ci in range(NCHUNK):
                        nc.tensor.transpose(
                            ps_xt[:, ci * 128:(ci + 1) * 128],
                            in_=xa[:, ci, l * 128:(l + 1) * 128],
                            identity=i128_sb,
                        )
                    if l % 2 == 0:
                        nc.vector.tensor_copy(xcT[:, t, :], ps_xt)
                    else:
                        nc.scalar.activation(
                            out=xcT[:, t, :], in_=ps_xt,
                            func=mybir.ActivationFunctionType.Copy,
                        )

            sums = small.tile([H, 1], F32, tag="sums")
            nc.vector.reduce_sum(sums, partials, axis=mybir.AxisListType.X)
            rsum = small.tile([H, 1], F32, tag="rsum")
            nc.vector.reciprocal(rsum, sums)

            # transpose attn -> attnT[n, h] chunks (PE transpose via identity);
            # the PSUM->SBUF copy also casts to bf16 for the phase-C matmul
            attnT = attn_pool.tile([128, N // 128, H], BF16, tag="attnT")
            for nn in range(N // 128):
                ps_t = ps_misc.tile([128, H], F32, tag="misc")
                nc.tensor.transpose(
                    ps_t, in_=attn[:, nn * 128:(nn + 1) * 128], identity=i12_sb)
                nc.vector.tensor_copy(attnT[:, nn, :], ps_t)

            # phase C: U[h, c] = sum_n attnT[n, h] * x[n, c].
            # SBUF-resident (transposed) chunks first: they are ready as soon
            # as attnT is, and the tail never waits on the xc DMA stream.
            psU0 = ps_c.tile([H, 384], F32, tag="psC0")
            psU1 = ps_c.tile([H, 384], F32, tag="psC1")
            psU = [psU0, psU1]
            for t in range(TCH):
                nn = (t // TPW) * CPW + (t % TPW)
                for j in range(2):
                    nc.tensor.matmul(
                        psU[j],
                        lhsT=attnT[:, nn, :],
                        rhs=xcT[:, t, j * 384:(j + 1) * 384],
                        start=(t == 0), stop=False,
                    )
            for w in range(NWIN):
                xc = xc_pool.tile([128, DPW, C], BF16, tag="xc")
                # issue phase-C loads on the other HWDGE engine so the two
                # x streams ride independent DMA queues
                nc.scalar.dma_start(
                    out=xc,
                    in_=x[b, w * NTW + TPW * 128:(w + 1) * NTW, :]
                         .rearrange("(t p) c -> p t c", p=128),
                )
                for d in range(DPW):
                    nn = w * CPW + TPW + d
                    last = (w == NWIN - 1) and (d == DPW - 1)
                    for j in range(2):
                        nc.tensor.matmul(
                            psU[j],
                            lhsT=attnT[:, nn, :],
                            rhs=xc[:, d, j * 384:(j + 1) * 384],
                            start=False, stop=last,
                        )
            # normalize by softmax sum while moving PSUM -> SBUF
            U_sb = small.tile([H, C], F32, tag="U")
            for j in range(2):
                nc.vector.tensor_scalar_mul(
                    out=U_sb[:, j * 384:(j + 1) * 384], in0=psU[j], scalar1=rsum,
                )

            # transpose U -> UT[c, h] chunks for the output projections
            for k in range(NCHUNK):
                ps_t = ps_misc.tile([128, H], F32, tag="misc")
                nc.tensor.transpose(ps_t, in_=U_sb[:, k * 128:(k + 1) * 128],
                                    identity=i12_sb)
                nc.vector.tensor_copy(ut_all[:, k, b, :], ps_t)

        # ---- P4a: ypre[h*64+d, b] = sum_c wvT[c, h*64+d] * UT[c, b, h] ----
        # these ride the sync queue, which is idle after the last xa tile
        wvT_sb = consts.tile([128, NCHUNK, C], BF16, tag="wvT_sb")
        nc.sync.dma_start(out=wvT_sb, in_=wvT.rearrange("(a p) o -> p a o", p=128))
        wpT_sb = consts.tile([128, NCHUNK, C], BF16, tag="wpT_sb")
        nc.sync.dma_start(out=wpT_sb, in_=wpT.rearrange("(a p) o -> p a o", p=128))
        ypT_sb = consts.tile([128, NCHUNK, BSH], BF16)
        for h in range(H):
            ps_yp = ps_misc.tile([64, BSH], F32, tag="misc")
            for k in range(NCHUNK):
                nc.tensor.matmul(
                    ps_yp,
                    lhsT=wvT_sb[:, k, h * 64:(h + 1) * 64],
                    rhs=ut_all[:, k, :, h],
                    start=(k == 0), stop=(k == NCHUNK - 1),
                )
            po = (h % 2) * 64
            nc.vector.tensor_copy(ypT_sb[po:po + 64, h // 2, :], ps_yp)

        # ---- P4b: y[b, c_out] = sum_c ypT[c, b] * wpT[c, c_out] + bp ----
        y_sb = small.tile([BSH, C], F32, tag="y")
        for j in range(2):
            ps_y = ps_misc.tile([BSH, 384], F32, tag="misc")
            for k in range(NCHUNK):
                nc.tensor.matmul(
                    ps_y,
                    lhsT=ypT_sb[:, k, :],
                    rhs=wpT_sb[:, k, j * 384:(j + 1) * 384],
                    start=(k == 0), stop=(k == NCHUNK - 1),
                )
            nc.vector.tensor_add(
                out=y_sb[:, j * 384:(j + 1) * 384],
                in0=ps_y,
                in1=bp_sb[:, j * 384:(j + 1) * 384],
            )
        nc.sync.dma_start(out=y, in_=y_sb)


_CACHE = {}
_BF16 = mybir.dt.np(mybir.dt.bfloat16)


def kernel(x, wq, wk, wv, wp, bp, trace=False):
    x = np.ascontiguousarray(x, dtype=np.float32)
    wq = np.asarray(wq, dtype=np.float32)
    wk = np.asarray(wk, dtype=np.float32)
    wv = np.asarray(wv, dtype=np.float32)
    wp = np.asarray(wp, dtype=np.float32)
    bp = np.asarray(bp, dtype=np.float32)

    if "nc" not in _CACHE:
        _CACHE["nc"] = build_kernel()
    nc = _CACHE["nc"]

    x_sh = x.reshape(NCORES, BSH, N, C)
    wqT = np.ascontiguousarray(wq.T.astype(_BF16))
    wkn = np.ascontiguousarray(wk.astype(_BF16))
    wvT = np.ascontiguousarray(wv.T.astype(_BF16))
    wpT = np.ascontiguousarray(wp.T.astype(_BF16))
    bp2 = np.ascontiguousarray(bp.reshape(1, C))
    i12 = np.eye(H, dtype=np.float32)
    i128 = np.eye(128, dtype=np.float32).astype(_BF16)

    in_maps = []
    for k in range(NCORES):
        xs = x_sh[k]
        in_maps.append({
            "xT": np.ascontiguousarray(xs.transpose(0, 2, 1).astype(_BF16)),
            "x": np.ascontiguousarray(xs.astype(_BF16)),
            "x0T": np.ascontiguousarray((xs[:, 0, :] * SCALE).T.astype(_BF16)),
            "wqT": wqT,
            "wk": wkn,
            "wvT": wvT,
            "wpT": wpT,
            "bp": bp2,
            "i12": i12,
            "i128": i128,
        })

    res = run_bass_kernel_spmd(nc, in_maps, core_ids=list(range(NCORES)),
                               trace=trace)
    out = np.concatenate([res.results[k]["y"] for k in range(NCORES)], axis=0)
    out = out.reshape(B, 1, C).astype(np.float32)
    if trace:
        _CACHE["last_exec_time_ns"] = res.exec_time_ns
        _CACHE["last_results"] = res
    return out
